# revision 4
# baseline (speedup 1.0000x reference)
"""MoE grouped-GEMM kernel for Trainium2 (8 NeuronCores, expert-parallel).

Problem: x [16384, 1024] fp16, expert_indices [16384] int32 (0..7),
weights [8, 1024, 4096] fp16. Output: fp16 [16384, 4096] in sorted-token
order (stable sort by expert), fp32 accumulation.

Sharding: the host performs the argsort/bincount dispatch (that IS the
sharding step) and gives core e the tokens routed to expert e as a
pre-transposed xT [K, Mpad] fp16 block plus that expert's weights
[K, N]. Every core runs the identical dense-GEMM program (token counts
padded to a common multiple of 128), so a single SPMD NEFF drives all 8
cores with no device-side collectives. The host concatenates the
per-expert output blocks, which is exactly sorted-token order.
"""

import numpy as np

_NCORES = 8


def _build_program(T, K, N):
    """Dense GEMM per core: out[Mpad, N] = xT.T @ w, fp32 PSUM accumulation.

    Layout per core:
      xT [K, Mpad] fp16  (x pre-transposed on host so K lands on partitions)
      w  [K, N]   fp16
      out [Mpad, N] fp16, Mpad = T*128

    PE mapping: stationary lhsT = xT k-tile [128, 128], moving rhs = w
    [128, 512] slice, PSUM pieces of [128, 1024] fp32 (2 banks, bufs=4)
    accumulated over K/128 k-tiles; DVE casts each piece to fp16 and the
    sync/scalar rails alternate stores.

    Ramp: the first RT tiles of the h=0 half are processed JOINTLY in two
    1024-col phases, so each fresh w chunk arriving from HBM feeds RT
    matmuls instead of one — the first pass is the only phase where every
    matmul needs first-use weight bytes, and with one tile per pass the PE
    outruns the input rail. Input DMAs are split across the sync and
    scalar rails in first-use order to double ramp delivery bandwidth.
    """
    from concourse import bacc, bass, tile
    import concourse.mybir as mybir
    from concourse.vector_clock import ScopedClock

    class _FastExitTC(tile.TileContext):
        # The stock exit path is drain -> barrier -> sem clears ->
        # barrier (~5us). The clears and second barrier only matter if
        # the NEFF is re-executed with warm semaphore state; this kernel
        # compiles a fresh NEFF per call and executes it once, so end
        # after the first barrier.
        def _drain_and_barrier(self, tick_clock, wait_clock):
            drain_inst = self.nc.sync.drain()
            wait_clock.add_sem_waits(
                drain_inst.ins, ScopedClock({None: tick_clock.global_clock})
            )
            self.nc.all_engine_barrier()
            popped = self.nc._tile_sem_poison_stack.pop()
            assert popped is self._sem_poison

    f16 = mybir.dt.float16
    f32 = mybir.dt.float32
    Mpad = T * 128
    KT = K // 128            # k-tiles (contraction)
    NB = 512                 # matmul moving-tile width (one PSUM bank fp32)
    PW = 1024                # psum piece width (2 banks)
    NH = 2048                # w h-strip width (4 banks worth of columns)
    nhalves = N // NH

    # Skip the ctor-time all-engine barrier (~3.4us of engine-arrival
    # stagger plus serialization before the first DMA can issue). All
    # cross-engine ordering in this kernel goes through semaphores, which
    # the runtime zeroes at NEFF load, and the NEFF runs exactly once per
    # compile — the barrier only guards warm-state reuse. The patch is
    # restored before TileContext exit, which still emits its barrier.
    _orig_aeb = bass.Bass.all_engine_barrier
    bass.Bass.all_engine_barrier = lambda self, *a, **k: None
    try:
        nc = bacc.Bacc(
            "TRN2",
            target_bir_lowering=False,
            debug=False,
            num_devices=_NCORES,
            # pure data-parallel SPMD: no instruction reads the core id
            enable_partition_id=False,
        )
    finally:
        bass.Bass.all_engine_barrier = _orig_aeb
    xT = nc.dram_tensor("xT", [K, Mpad], f16, kind="ExternalInput").ap()
    w = nc.dram_tensor("w", [K, N], f16, kind="ExternalInput").ap()
    out = nc.dram_tensor("out", [Mpad, N], f16, kind="ExternalOutput").ap()

    RT = min(4, T)           # tiles processed jointly during the ramp
    TE = min(6, T)           # tiles covered by ramp strips + early rest

    with _FastExitTC(nc) as tc:
        with (
            tc.tile_pool(name="xw", bufs=1) as xw,
            tc.tile_pool(name="op", bufs=6) as op,
            tc.tile_pool(name="pp", bufs=4, space=bass.MemorySpace.PSUM) as pp,
        ):
            # Whole x and w stay SBUF-resident (~110KB/partition total).
            # PE clock-gate warm-up: matmuls on memset tiles issued
            # during the initial DMA wait so the HAM un-throttles (1.2 ->
            # 2.4GHz takes ~3.4us of sustained PE activity) before the
            # first real matmul. gpsimd exits the entry butterfly
            # earliest, so its memsets unblock the dummy burst soonest.
            zs = xw.tile([128, 128], f16, tag="zstat")
            zm = xw.tile([128, NB], f16, tag="zmov")
            nc.gpsimd.memset(zs[:], 0.0)
            nc.gpsimd.memset(zm[:], 0.0)
            pwarm = pp.tile([128, PW], f32, tag="ps")
            for i in range(7):
                nc.tensor.matmul(
                    pwarm[:, 0:NB], zs[:], zm[:], start=(i == 0), stop=(i == 6)
                )

            # Input DMAs, split across both rails in exact first-use
            # order. Sync rail: per-k (x ramp strip, then the phase-A w
            # half) so the first matmuls' deps are shallow, then the late
            # x strips, then the h=1 w strips. Scalar rail: the phase-B w
            # halves (first used ~10us in), then the t in [RT, TE) x
            # strips. Output stores only start needing the rails ~17us in.
            xheads = []
            xearly = []
            xlate = []
            ws1 = [None] * KT        # h=1 strips [128, NH]
            wA = [None] * KT         # h=0 cols [0, PW)
            wB = [None] * KT         # h=0 cols [PW, NH)
            w0c = []                 # k=0 phase-A chunked into NB pieces
            for k in range(KT):
                xh = xw.tile([128, RT * 128], f16, tag=f"xh{k}")
                nc.sync.dma_start(xh[:], xT[k * 128 : (k + 1) * 128, 0 : RT * 128])
                xheads.append(xh)
                if k == 0:
                    for n in range(PW // NB):
                        c = xw.tile([128, NB], f16, tag=f"w0c{n}")
                        nc.sync.dma_start(c[:], w[0:128, n * NB : (n + 1) * NB])
                        w0c.append(c)
                else:
                    wt = xw.tile([128, PW], f16, tag=f"wA{k}")
                    nc.sync.dma_start(wt[:], w[k * 128 : (k + 1) * 128, 0:PW])
                    wA[k] = wt
            for k in range(KT):
                wt = xw.tile([128, PW], f16, tag=f"wB{k}")
                nc.scalar.dma_start(wt[:], w[k * 128 : (k + 1) * 128, PW:NH])
                wB[k] = wt
            if T > TE:
                for k in range(KT):
                    xl = xw.tile([128, (T - TE) * 128], f16, tag=f"xl{k}")
                    nc.sync.dma_start(
                        xl[:], xT[k * 128 : (k + 1) * 128, TE * 128 : Mpad]
                    )
                    xlate.append(xl)
            if TE > RT:
                for k in range(KT):
                    xe = xw.tile([128, (TE - RT) * 128], f16, tag=f"xe{k}")
                    nc.scalar.dma_start(
                        xe[:], xT[k * 128 : (k + 1) * 128, RT * 128 : TE * 128]
                    )
                    xearly.append(xe)
            for h in range(1, nhalves):
                for k in range(KT):
                    wt = xw.tile([128, NH], f16, tag=f"w{k}h{h}")
                    nc.sync.dma_start(
                        wt[:], w[k * 128 : (k + 1) * 128, h * NH : (h + 1) * NH]
                    )
                    ws1[k] = wt

            def lhs_for(k, t):
                if t < RT:
                    return xheads[k][:, t * 128 : (t + 1) * 128]
                if t < TE:
                    return xearly[k][:, (t - RT) * 128 : (t - RT + 1) * 128]
                return xlate[k][:, (t - TE) * 128 : (t - TE + 1) * 128]

            def rhs_for(k, h, n0):
                # n0 is the 512-col slice index within the h-half
                if h == 0:
                    if k == 0 and n0 < PW // NB:
                        return w0c[n0][:]
                    half, n = (wA, n0) if n0 < PW // NB else (wB, n0 - PW // NB)
                    return half[k][:, n * NB : (n + 1) * NB]
                return ws1[k][:, n0 * NB : (n0 + 1) * NB]

            # Output chunks alternate rails (either alone barely keeps
            # up); parity arranged so the very last chunk — which gates
            # the exit drain — rides the faster sync rail.
            # pieces: ramp 2*RT + steady (T-RT)*2 for h0 + T*2*(nhalves-1);
            # the last piece is stored as two chunks
            n_pieces = 2 * RT + (T - RT) * 2 + T * 2 * (nhalves - 1)
            n_chunks = n_pieces + 1
            chunk_i = [0]

            def store(ps, t, col0, width, nq):
                for q in range(width // nq):
                    ot = op.tile([128, PW], f16, tag="ot")
                    nc.vector.tensor_copy(
                        ot[:, :nq], ps[:, q * nq : (q + 1) * nq]
                    )
                    eng = (
                        nc.sync
                        if (n_chunks - 1 - chunk_i[0]) % 2 == 0
                        else nc.scalar
                    )
                    chunk_i[0] += 1
                    c0 = col0 + q * nq
                    eng.dma_start(
                        out[t * 128 : (t + 1) * 128, c0 : c0 + nq], ot[:, :nq]
                    )

            # Ramp: tiles 0..RT-1 of h=0 jointly, two PW-wide phases.
            for p in range(NH // PW):
                pss = [
                    pp.tile([128, PW], f32, tag="ps", name=f"psr{p}_{i}")
                    for i in range(RT)
                ]
                for k in range(KT):
                    for n in range(PW // NB):
                        rhs = rhs_for(k, 0, p * (PW // NB) + n)
                        for i in range(RT):
                            nc.tensor.matmul(
                                pss[i][:, n * NB : (n + 1) * NB],
                                lhs_for(k, i),
                                rhs,
                                start=(k == 0),
                                stop=(k == KT - 1),
                            )
                for i in range(RT):
                    store(pss[i], i, p * PW, PW, PW)

            # Steady state: one PW piece at a time, 4-deep psum ring.
            for h in range(nhalves):
                ts = range(RT, T) if h == 0 else range(T)
                for t in ts:
                    for p in range(NH // PW):
                        last = (
                            h == nhalves - 1 and t == T - 1 and p == NH // PW - 1
                        )
                        ps = pp.tile([128, PW], f32, tag="ps")
                        for k in range(KT):
                            lhs = lhs_for(k, t)
                            for n in range(PW // NB):
                                nc.tensor.matmul(
                                    ps[:, n * NB : (n + 1) * NB],
                                    lhs,
                                    rhs_for(k, h, p * (PW // NB) + n),
                                    start=(k == 0),
                                    stop=(k == KT - 1),
                                )
                        # the final piece is stored in two NB chunks so
                        # the kernel tail (cast + store of the last
                        # columns) is half as deep
                        store(ps, t, h * NH + p * PW, PW, NB if last else PW)
    nc.compile()
    return nc


# test.py reads these after a call for timing/trace introspection
last_results = None


def kernel(x, expert_indices, weights):
    x = np.asarray(x)
    ei = np.asarray(expert_indices)
    w = np.asarray(weights)
    M, K = x.shape
    E, K2, N = w.shape
    assert K == K2 and E == _NCORES

    counts = np.bincount(ei, minlength=E)
    T = max(1, -(-int(counts.max()) // 128))
    Mpad = T * 128
    order = np.argsort(ei, kind="stable")
    x_sorted = x[order]
    offs = np.zeros(E + 1, dtype=np.int64)
    np.cumsum(counts, out=offs[1:])

    in_maps = []
    for e in range(E):
        blk = x_sorted[offs[e] : offs[e + 1]]
        xeT = np.zeros((K, Mpad), dtype=np.float16)
        xeT[:, : blk.shape[0]] = blk.T
        in_maps.append({"xT": xeT, "w": np.ascontiguousarray(w[e])})

    nc = _build_program(T, K, N)

    from concourse.bass_utils import run_bass_kernel_spmd

    res = run_bass_kernel_spmd(nc, in_maps, list(range(E)))
    global last_results
    last_results = res

    out = np.empty((M, N), dtype=np.float16)
    for e in range(E):
        out[offs[e] : offs[e + 1]] = res.results[e]["out"][: counts[e]]
    return out
